# revision 1
# baseline (speedup 1.0000x reference)
"""Causal self-attention (B=2, S=2048, E=2048, H=16) on 8 TRN2 NeuronCores.

Sharding: 2-way batch x 4-way head-group tensor parallel.
Core c handles batch c//4 and heads [4*(c%4), 4*(c%4)+4).

Per-core kernel:
  phase 1: X^T via PE transposes; X arrives host-split into bf16 hi/lo
  phase 2: QKV projection as 3 exact bf16 products (hi*hi + hi*lo + lo*hi,
           ~fp32 precision at 3 cyc/row instead of fp32's 4 cyc/row)
           -> feature-major qT/kT/vT [128hd, S] fp32, staged through DRAM
  phase 3: per head: causal attention (fp32 q-major scores, chunked exp with
           accumulated row-sums, P normalized then PE-transposed, PV)
  phase 4: out projection, attT chip-split to bf16 hi/lo, W_out host-split

Host side: shard + bf16-split inputs, run SPMD on 8 cores, sum the 4
head-group partials per batch and add (b_out + b_v @ W_out) once.
"""

from contextlib import ExitStack

import ml_dtypes
import numpy as np

import concourse.bass as bass
import concourse.tile as tile
from concourse import bacc, bass_utils, mybir
from concourse.masks import make_causal_mask, make_identity

FP = mybir.dt.float32
BF = mybir.dt.bfloat16
AF = mybir.ActivationFunctionType

B = 2
S = 2048
E = 2048
H = 16
HD = 128
NCORES = 8
HG = 4  # head-group axis (tensor parallel)
H_LOC = H // HG  # 4 heads per core
FLOC = H_LOC * HD  # 512 local features per q/k/v
SCALE = 1.0 / float(np.sqrt(HD))
NEG = -1.0e30

PROFILE = False
LAST_EXEC_NS = None
LAST_RESULTS = None


def _emit(nc, S=S, E=E):
    NB = S // 128
    EB = E // 128
    xh = nc.dram_tensor("xh", [S, E], BF, kind="ExternalInput").ap()
    xl = nc.dram_tensor("xl", [S, E], BF, kind="ExternalInput").ap()
    wqkv_hl = []
    for wn in ("wq", "wk", "wv"):
        pair = []
        for p in ("h", "l"):
            pair.append(
                nc.dram_tensor(f"{wn}{p}", [E, FLOC], BF, kind="ExternalInput").ap()
            )
        wqkv_hl.append(pair)
    bqs = nc.dram_tensor("bqs", [FLOC, 1], FP, kind="ExternalInput").ap()  # *SCALE
    bk = nc.dram_tensor("bk", [FLOC, 1], FP, kind="ExternalInput").ap()
    woh = nc.dram_tensor("woh", [FLOC, E], BF, kind="ExternalInput").ap()
    wol = nc.dram_tensor("wol", [FLOC, E], BF, kind="ExternalInput").ap()
    out = nc.dram_tensor("out", [S, E], FP, kind="ExternalOutput").ap()

    with tile.TileContext(nc) as tc, ExitStack() as top:
        dram = top.enter_context(tc.tile_pool(name="dram", bufs=1, space="DRAM"))
        # feature-major fp32 scratch: per head 128 rows (hd) x S cols
        qT = [dram.tile([128, S], FP, name=f"qT{h}", tag=f"qT{h}") for h in range(H_LOC)]
        kT = [dram.tile([128, S], FP, name=f"kT{h}", tag=f"kT{h}") for h in range(H_LOC)]
        vT = [dram.tile([128, S], FP, name=f"vT{h}", tag=f"vT{h}") for h in range(H_LOC)]
        qkvT = [qT, kT, vT]

        cst = top.enter_context(tc.tile_pool(name="cst", bufs=1))
        ident = cst.tile([128, 128], FP, name="ident", tag="ident")
        make_identity(nc, ident[:])
        ident_bf = cst.tile([128, 128], BF, name="identbf", tag="identbf")
        make_identity(nc, ident_bf[:])
        cmask = cst.tile([128, 128], FP, name="cmask", tag="cmask")
        make_causal_mask(nc, cmask[:], mask_val=NEG)
        bq_sb = cst.tile([128, H_LOC], FP, name="bq", tag="bq")
        bk_sb = cst.tile([128, H_LOC], FP, name="bk", tag="bk")
        for f in range(H_LOC):
            nc.sync.dma_start(bq_sb[:, f : f + 1], bqs[128 * f : 128 * (f + 1), :])
            nc.sync.dma_start(bk_sb[:, f : f + 1], bk[128 * f : 128 * (f + 1), :])

        ps_aux = top.enter_context(tc.tile_pool(name="ps_aux", bufs=4, space="PSUM"))

        # ---------------- phase 1+2: X^T and QKV projection ----------------
        with ExitStack() as ph, nc.named_scope("proj"):
            xt_pool = ph.enter_context(tc.tile_pool(name="xt", bufs=1))
            xts = [
                [
                    xt_pool.tile([128, S], BF, name=f"xt{p}{j}", tag=f"xt{p}{j}")
                    for j in range(EB)
                ]
                for p in range(2)  # 0=hi, 1=lo
            ]
            xin = ph.enter_context(tc.tile_pool(name="xin", bufs=4))
            ps_main = ph.enter_context(
                tc.tile_pool(name="ps_main", bufs=4, space="PSUM")
            )
            wpool = ph.enter_context(tc.tile_pool(name="w", bufs=4))
            stg = ph.enter_context(tc.tile_pool(name="stg", bufs=4))

            # X^T for hi and lo parts
            for p, xsrc in enumerate((xh, xl)):
                for ig in range(NB // 4):
                    xrow = []
                    for m in range(4):
                        i = 4 * ig + m
                        xr = xin.tile([128, E], BF, name="xin", tag="xin")
                        nc.sync.dma_start(xr[:], xsrc[128 * i : 128 * (i + 1), :])
                        xrow.append(xr)
                    for j in range(EB):
                        pt = ps_main.tile([128, 512], BF, name="psb", tag="ps")
                        for m in range(4):
                            nc.tensor.transpose(
                                pt[:, 128 * m : 128 * (m + 1)],
                                xrow[m][:, 128 * j : 128 * (j + 1)],
                                ident_bf[:],
                            )
                        nc.scalar.activation(
                            xts[p][j][:, 512 * ig : 512 * (ig + 1)], pt[:], AF.Copy
                        )

            # projections: 3 bf16 products, f-block == (which, head)
            for which in range(3):  # q, k, v
                wth_d, wtl_d = wqkv_hl[which]
                for h in range(H_LOC):
                    nsc = S // 512
                    psums = []
                    for sc in range(nsc):
                        psums.append(ps_main.tile([128, 512], FP, name="ps", tag="ps"))
                    for e in range(EB):
                        wth = wpool.tile([128, 128], BF, name="wh", tag="wh")
                        nc.sync.dma_start(
                            wth[:],
                            wth_d[128 * e : 128 * (e + 1), 128 * h : 128 * (h + 1)],
                        )
                        wtl = wpool.tile([128, 128], BF, name="wl", tag="wl")
                        nc.sync.dma_start(
                            wtl[:],
                            wtl_d[128 * e : 128 * (e + 1), 128 * h : 128 * (h + 1)],
                        )
                        first = e == 0
                        last = e == EB - 1
                        for sc in range(nsc):
                            sl = slice(512 * sc, 512 * (sc + 1))
                            nc.tensor.matmul(
                                psums[sc][:], wth[:], xts[0][e][:, sl],
                                start=first, stop=False,
                            )
                            nc.tensor.matmul(
                                psums[sc][:], wth[:], xts[1][e][:, sl],
                                start=False, stop=False,
                            )
                            nc.tensor.matmul(
                                psums[sc][:], wtl[:], xts[0][e][:, sl],
                                start=False, stop=last,
                            )
                    for sc in range(nsc):
                        st = stg.tile([128, 512], FP, name="stg", tag="stg")
                        if which == 0:
                            nc.vector.tensor_scalar(
                                st[:], psums[sc][:], SCALE, bq_sb[:, h : h + 1],
                                op0=mybir.AluOpType.mult, op1=mybir.AluOpType.add,
                            )
                        elif which == 1:
                            nc.vector.tensor_scalar_add(
                                st[:], psums[sc][:], bk_sb[:, h : h + 1]
                            )
                        else:
                            nc.scalar.activation(st[:], psums[sc][:], AF.Copy)
                        nc.sync.dma_start(
                            qkvT[which][h][:, 512 * sc : 512 * (sc + 1)], st[:]
                        )

        # ---------------- phase 3: attention per head ----------------
        with ExitStack() as ao:
            att_pool = ao.enter_context(tc.tile_pool(name="att", bufs=1))
            attT = []  # (hi, lo) bf16 pairs
            with ExitStack() as ph:
                qkv_pool = ph.enter_context(tc.tile_pool(name="qkv", bufs=2))
                vsb_pool = ph.enter_context(tc.tile_pool(name="vsb", bufs=2))
                p_pool = ph.enter_context(tc.tile_pool(name="p", bufs=2))
                pt_pool = ph.enter_context(tc.tile_pool(name="pt", bufs=2))
                rs_pool = ph.enter_context(tc.tile_pool(name="rs", bufs=4))
                ps_sc = ph.enter_context(
                    tc.tile_pool(name="ps_sc", bufs=4, space="PSUM")
                )

                for h in range(H_LOC):
                    with nc.named_scope(f"attn{h}"):
                        qt = qkv_pool.tile([128, S], FP, name="qt", tag="qt")
                        kt = qkv_pool.tile([128, S], FP, name="kt", tag="kt")
                        vt = qkv_pool.tile([128, S], FP, name="vt", tag="vt")
                        nc.sync.dma_start(qt[:], qT[h][:])
                        nc.sync.dma_start(kt[:], kT[h][:])
                        nc.sync.dma_start(vt[:], vT[h][:])

                        # V -> token-major [s-block, hd] tiles
                        vsb = vsb_pool.tile([128, S], FP, name="vsb", tag="vsb")
                        for mg in range(NB // 4):
                            pv = ps_aux.tile([128, 512], FP, name="psa", tag="psa")
                            for m in range(4):
                                nc.tensor.transpose(
                                    pv[:, 128 * m : 128 * (m + 1)],
                                    vt[:, 128 * (4 * mg + m) : 128 * (4 * mg + m + 1)],
                                    ident[:],
                                )
                            nc.scalar.activation(
                                vsb[:, 512 * mg : 512 * (mg + 1)], pv[:], AF.Copy
                            )

                        att_h = att_pool.tile(
                            [128, S], BF, name=f"atth{h}", tag=f"atth{h}"
                        )
                        att_l = att_pool.tile(
                            [128, S], BF, name=f"attl{h}", tag=f"attl{h}"
                        )
                        attT.append((att_h, att_l))

                        for g in range(S // 512):  # q-groups of 512
                            PT = pt_pool.tile([128, 4 * S], FP, name="PT", tag="PT")
                            nkc = 4 * (g + 1)  # key chunks of 128 for this group
                            for qs in range(4):
                                i = 4 * g + qs  # q-block
                                L = 128 * (i + 1)
                                nq0 = 128 * i
                                p = p_pool.tile([128, 2048], FP, name="p", tag="p")
                                rs = rs_pool.tile([128, 6], FP, name="rs", tag="rs")
                                ncchunks = (L + 511) // 512
                                for c in range(ncchunks):
                                    w = min(512, L - 512 * c)
                                    psc = ps_sc.tile(
                                        [128, 512], FP, name="psc", tag="psc"
                                    )
                                    nc.tensor.matmul(
                                        psc[:, :w],
                                        qt[:, nq0 : nq0 + 128],
                                        kt[:, 512 * c : 512 * c + w],
                                        start=True,
                                        stop=True,
                                    )
                                    if c == ncchunks - 1:
                                        # causal mask on the diagonal 128 cols
                                        nc.vector.tensor_add(
                                            psc[:, w - 128 : w],
                                            psc[:, w - 128 : w],
                                            cmask[:],
                                        )
                                    nc.scalar.activation(
                                        p[:, 512 * c : 512 * c + w], psc[:, :w],
                                        AF.Exp, accum_out=rs[:, c : c + 1],
                                    )
                                for c in range(1, ncchunks):
                                    nc.vector.tensor_add(
                                        rs[:, 0:1], rs[:, 0:1], rs[:, c : c + 1]
                                    )
                                nc.vector.reciprocal(rs[:, 4:5], rs[:, 0:1])
                                nc.vector.tensor_scalar_mul(
                                    p[:, :L], p[:, :L], rs[:, 4:5]
                                )
                                # transpose P into PT (k-major)
                                for jg in range((i + 1 + 3) // 4):
                                    nm = min(4, i + 1 - 4 * jg)
                                    ptp = ps_aux.tile(
                                        [128, 512], FP, name="psa", tag="psa"
                                    )
                                    for m in range(nm):
                                        j = 4 * jg + m
                                        nc.tensor.transpose(
                                            ptp[:, 128 * m : 128 * (m + 1)],
                                            p[:, 128 * j : 128 * (j + 1)],
                                            ident[:],
                                        )
                                    src = ptp[:, : 128 * nm].rearrange(
                                        "p (m q) -> p m q", q=128
                                    )
                                    dst = PT.rearrange("p (j q) -> p j q", q=512)[
                                        :, 4 * jg : 4 * jg + nm,
                                        128 * qs : 128 * (qs + 1),
                                    ]
                                    nc.scalar.activation(dst, src, AF.Copy)
                            # PV for the group
                            po = ps_aux.tile([128, 512], FP, name="psa", tag="psa")
                            for j in range(nkc):
                                qlo = max(0, 128 * (j - 4 * g))  # causal: q >= k
                                nc.tensor.matmul(
                                    po[:, qlo:512],
                                    vsb[:, 128 * j : 128 * (j + 1)],
                                    PT[:, 512 * j + qlo : 512 * j + 512],
                                    start=(j == 0),
                                    stop=(j == nkc - 1),
                                )
                            gsl = slice(512 * g, 512 * (g + 1))
                            nc.scalar.activation(att_h[:, gsl], po[:], AF.Copy)
                            nc.vector.tensor_sub(att_l[:, gsl], po[:], att_h[:, gsl])

            # ---------------- phase 4: output projection ----------------
            with ExitStack() as ph, nc.named_scope("outproj"):
                wo_pool = ph.enter_context(tc.tile_pool(name="wo", bufs=1))
                ostg = ph.enter_context(tc.tile_pool(name="ostg", bufs=4))
                ps_out = ph.enter_context(
                    tc.tile_pool(name="ps_out", bufs=4, space="PSUM")
                )
                wohs, wols = [], []
                for h in range(H_LOC):
                    wt = wo_pool.tile([128, E], BF, name=f"woh{h}", tag=f"woh{h}")
                    nc.sync.dma_start(wt[:], woh[128 * h : 128 * (h + 1), :])
                    wohs.append(wt)
                    wt = wo_pool.tile([128, E], BF, name=f"wol{h}", tag=f"wol{h}")
                    nc.sync.dma_start(wt[:], wol[128 * h : 128 * (h + 1), :])
                    wols.append(wt)
                nec = E // 512
                for i in range(NB):
                    psums = []
                    for c in range(nec):
                        psums.append(
                            ps_out.tile([128, 512], FP, name="pso", tag="pso")
                        )
                    for h in range(H_LOC):
                        ah = attT[h][0][:, 128 * i : 128 * (i + 1)]
                        al = attT[h][1][:, 128 * i : 128 * (i + 1)]
                        first = h == 0
                        last = h == H_LOC - 1
                        for c in range(nec):
                            sl = slice(512 * c, 512 * (c + 1))
                            nc.tensor.matmul(
                                psums[c][:], ah, wohs[h][:, sl],
                                start=first, stop=False,
                            )
                            nc.tensor.matmul(
                                psums[c][:], ah, wols[h][:, sl],
                                start=False, stop=False,
                            )
                            nc.tensor.matmul(
                                psums[c][:], al, wohs[h][:, sl],
                                start=False, stop=last,
                            )
                    for c in range(nec):
                        ot = ostg.tile([128, 512], FP, name="ostg", tag="ostg")
                        nc.scalar.activation(ot[:], psums[c][:], AF.Copy)
                        nc.sync.dma_start(
                            out[128 * i : 128 * (i + 1), 512 * c : 512 * (c + 1)],
                            ot[:],
                        )


_NC_CACHE = None


def _get_nc():
    global _NC_CACHE
    if _NC_CACHE is None:
        nc = bacc.Bacc(
            "TRN2",
            target_bir_lowering=False,
            debug=False,
            num_devices=1,
            enable_asserts=False,
        )
        _emit(nc)
        nc.compile()
        _NC_CACHE = nc
    return _NC_CACHE


def _split(a):
    hi = a.astype(ml_dtypes.bfloat16)
    lo = (a - hi.astype(np.float32)).astype(ml_dtypes.bfloat16)
    return hi, lo


def make_in_maps(inX, W_qkv, b_qkv, W_out):
    in_maps = []
    for c in range(NCORES):
        b = c // HG
        hg = c % HG
        sl = slice(FLOC * hg, FLOC * (hg + 1))
        xh_, xl_ = _split(inX[b])
        wqh_, wql_ = _split(W_qkv[:, 0:E][:, sl])
        wkh_, wkl_ = _split(W_qkv[:, E : 2 * E][:, sl])
        wvh_, wvl_ = _split(W_qkv[:, 2 * E : 3 * E][:, sl])
        woh_, wol_ = _split(W_out[sl, :])
        in_maps.append(
            {
                "xh": np.ascontiguousarray(xh_),
                "xl": np.ascontiguousarray(xl_),
                "wqh": np.ascontiguousarray(wqh_),
                "wql": np.ascontiguousarray(wql_),
                "wkh": np.ascontiguousarray(wkh_),
                "wkl": np.ascontiguousarray(wkl_),
                "wvh": np.ascontiguousarray(wvh_),
                "wvl": np.ascontiguousarray(wvl_),
                "bqs": np.ascontiguousarray(
                    (b_qkv[0:E][sl] * SCALE).reshape(FLOC, 1)
                ),
                "bk": np.ascontiguousarray(b_qkv[E : 2 * E][sl].reshape(FLOC, 1)),
                "woh": np.ascontiguousarray(woh_),
                "wol": np.ascontiguousarray(wol_),
            }
        )
    return in_maps


def kernel(inX, W_qkv, b_qkv, W_out, b_out):
    global LAST_EXEC_NS, LAST_RESULTS
    inX = np.asarray(inX, dtype=np.float32)
    W_qkv = np.asarray(W_qkv, dtype=np.float32)
    b_qkv = np.asarray(b_qkv, dtype=np.float32)
    W_out = np.asarray(W_out, dtype=np.float32)
    b_out = np.asarray(b_out, dtype=np.float32)

    nc = _get_nc()
    in_maps = make_in_maps(inX, W_qkv, b_qkv, W_out)

    kwargs = {}
    if PROFILE:
        kwargs = {"trace": True, "trace_cores": [0]}
    res = bass_utils.run_bass_kernel_spmd(
        nc, in_maps, core_ids=list(range(NCORES)), **kwargs
    )
    LAST_EXEC_NS = res.exec_time_ns
    LAST_RESULTS = res

    bias_full = (b_out + b_qkv[2 * E : 3 * E] @ W_out).astype(np.float32)
    out = np.empty((B, S, E), dtype=np.float32)
    for b in range(B):
        acc = res.results[HG * b + 0]["out"].astype(np.float64)
        for hg in range(1, HG):
            acc += res.results[HG * b + hg]["out"]
        out[b] = (acc + bias_full).astype(np.float32)
    return out



# revision 3
# speedup vs baseline: 2.3520x; 2.3520x over previous
"""Causal self-attention (B=2, S=2048, E=2048, H=16) on 8 TRN2 NeuronCores.

Sharding: 2-way batch x 4-way head-group tensor parallel.
Core c handles batch c//4 and heads [4*(c%4), 4*(c%4)+4).

Per-core kernel (all matmuls single-product bf16, fp32 PSUM accumulation;
the 2e-2 correctness gate leaves plenty of headroom vs fp32):
  phase 1: X^T via PE transposes (bf16), interleaved with V projection
           (token-major: lhsT = X^T block, rhs = W_v) so PE work covers
           the X DMA stream
  phase 2: Q,K projections feature-major (lhsT = W block, rhs = X^T),
           SCALE folded into W_q/b_q on host, bias added during the
           psum->SBUF copy on the vector engine
  phase 3: per head: causal attention: q-major scores (bf16, chunked
           1024-wide psum), exp on scalar engine with accumulated row
           sums, in-place normalize (vector), P^T via PE transposes
           (bf16 psum), PV per 512-query group
  phase 4: out projection (bf16), partial over this core's 4 heads
Host: shard + bf16-cast inputs, SPMD on 8 cores, sum the 4 head-group
partials per batch and add (b_out + b_v @ W_out) once.
"""

from contextlib import ExitStack

import ml_dtypes
import numpy as np

import concourse.bass as bass
import concourse.tile as tile
from concourse import bacc, bass_utils, mybir
from concourse.masks import make_causal_mask, make_identity

FP = mybir.dt.float32
BF = mybir.dt.bfloat16
AF = mybir.ActivationFunctionType

B = 2
S = 2048
E = 2048
H = 16
HD = 128
NCORES = 8
HG = 4  # head-group axis (tensor parallel)
H_LOC = H // HG  # 4 heads per core
FLOC = H_LOC * HD  # 512 local features per q/k/v
SCALE = 1.0 / float(np.sqrt(HD))
NEG = -1.0e30

NB = S // 128  # 16 token blocks
EB = E // 128  # 16 contraction blocks

PROFILE = False
LAST_EXEC_NS = None
LAST_RESULTS = None


def _emit(nc):
    xh = nc.dram_tensor("xh", [S, E], BF, kind="ExternalInput").ap()
    wq_d = nc.dram_tensor("wq", [E, FLOC], BF, kind="ExternalInput").ap()
    wk_d = nc.dram_tensor("wk", [E, FLOC], BF, kind="ExternalInput").ap()
    wv_d = nc.dram_tensor("wv", [E, FLOC], BF, kind="ExternalInput").ap()
    bqk_d = nc.dram_tensor("bqk", [128, 2 * H_LOC], FP, kind="ExternalInput").ap()
    wo_d = nc.dram_tensor("wo", [FLOC, E], BF, kind="ExternalInput").ap()
    out = nc.dram_tensor("out", [S, E], FP, kind="ExternalOutput").ap()

    with tile.TileContext(nc) as tc, ExitStack() as top:
        cst = top.enter_context(tc.tile_pool(name="cst", bufs=1))
        ident_bf = cst.tile([128, 128], BF, name="identbf", tag="identbf")
        make_identity(nc, ident_bf[:])
        cmask = cst.tile([128, 128], FP, name="cmask", tag="cmask")
        make_causal_mask(nc, cmask[:], mask_val=NEG)
        bqk = cst.tile([128, 2 * H_LOC], FP, name="bqk", tag="bqk")
        nc.sync.dma_start(bqk[:], bqk_d[:, :])

        # outputs of the projection phase, consumed by attention
        qkv_out = top.enter_context(tc.tile_pool(name="qkvo", bufs=1))
        qT = [qkv_out.tile([128, S], BF, name=f"qT{h}", tag=f"qT{h}")
              for h in range(H_LOC)]
        kT = [qkv_out.tile([128, S], BF, name=f"kT{h}", tag=f"kT{h}")
              for h in range(H_LOC)]
        # token-major V: for k-block j, head h: vsb[:, 512*j+128*h :][128, 128]
        vsb = qkv_out.tile([128, 4 * S], BF, name="vsb", tag="vsb")

        # PSUM pools: psF fp32 [128,1024] = 2 banks x 3 bufs = 6 banks,
        # psT bf16 [128,1024] = 1 bank x 2 bufs = 2 banks -> 8 total
        psF = top.enter_context(tc.tile_pool(name="psF", bufs=3, space="PSUM"))
        psT = top.enter_context(tc.tile_pool(name="psT", bufs=2, space="PSUM"))

        # ---------------- phase 1+2: X^T, V proj, Q/K proj ----------------
        with ExitStack() as ph, nc.named_scope("proj"):
            wpool = ph.enter_context(tc.tile_pool(name="w", bufs=1))
            wv_sb = [wpool.tile([128, FLOC], BF, name=f"wv{e}", tag=f"wv{e}")
                     for e in range(EB)]
            wq_sb = [wpool.tile([128, FLOC], BF, name=f"wq{e}", tag=f"wq{e}")
                     for e in range(EB)]
            wk_sb = [wpool.tile([128, FLOC], BF, name=f"wk{e}", tag=f"wk{e}")
                     for e in range(EB)]
            for e in range(EB):
                nc.sync.dma_start(wv_sb[e][:], wv_d[128 * e : 128 * (e + 1), :])
            xt_pool = ph.enter_context(tc.tile_pool(name="xt", bufs=1))
            xts = [xt_pool.tile([128, S], BF, name=f"xt{j}", tag=f"xt{j}")
                   for j in range(EB)]
            xin = ph.enter_context(tc.tile_pool(name="xin", bufs=4))

            for ig in range(NB // 4):
                # X^T of token blocks 4ig..4ig+3
                xrow = []
                for m in range(4):
                    i = 4 * ig + m
                    xr = xin.tile([128, E], BF, name="xin", tag="xin")
                    nc.sync.dma_start(xr[:], xh[128 * i : 128 * (i + 1), :])
                    xrow.append(xr)
                for j in range(EB):
                    pt = psT.tile([128, 1024], BF, name="pst", tag="pst")
                    for m in range(4):
                        nc.tensor.transpose(
                            pt[:, 128 * m : 128 * (m + 1)],
                            xrow[m][:, 128 * j : 128 * (j + 1)],
                            ident_bf[:],
                        )
                    nc.vector.tensor_copy(
                        xts[j][:, 512 * ig : 512 * (ig + 1)], pt[:, :512]
                    )
                # V projection for token blocks 4ig..4ig+3 (pairs)
                for m in range(0, 4, 2):
                    i = 4 * ig + m
                    ps = psF.tile([128, 1024], FP, name="psf", tag="psf")
                    for e in range(EB):
                        first = e == 0
                        last = e == EB - 1
                        nc.tensor.matmul(
                            ps[:, 0:512],
                            xts[e][:, 128 * i : 128 * (i + 1)],
                            wv_sb[e][:],
                            start=first, stop=last,
                        )
                        nc.tensor.matmul(
                            ps[:, 512:1024],
                            xts[e][:, 128 * (i + 1) : 128 * (i + 2)],
                            wv_sb[e][:],
                            start=first, stop=last,
                        )
                    nc.vector.tensor_copy(
                        vsb[:, 512 * i : 512 * (i + 2)], ps[:]
                    )
                if ig == 0:
                    for e in range(EB):
                        nc.sync.dma_start(
                            wq_sb[e][:], wq_d[128 * e : 128 * (e + 1), :]
                        )
                        nc.sync.dma_start(
                            wk_sb[e][:], wk_d[128 * e : 128 * (e + 1), :]
                        )

            # Q, K projections: feature-major [128 hd, S]
            for h in range(H_LOC):
                for which, wsb, dstl, bcol in (
                    (0, wq_sb, qT, h),
                    (1, wk_sb, kT, H_LOC + h),
                ):
                    for scp in range(S // 1024):
                        ps = psF.tile([128, 1024], FP, name="psf", tag="psf")
                        for e in range(EB):
                            wt = wsb[e][:, 128 * h : 128 * (h + 1)]
                            first = e == 0
                            last = e == EB - 1
                            nc.tensor.matmul(
                                ps[:, 0:512], wt,
                                xts[e][:, 1024 * scp : 1024 * scp + 512],
                                start=first, stop=last,
                            )
                            nc.tensor.matmul(
                                ps[:, 512:1024], wt,
                                xts[e][:, 1024 * scp + 512 : 1024 * (scp + 1)],
                                start=first, stop=last,
                            )
                        nc.vector.tensor_scalar_add(
                            dstl[h][:, 1024 * scp : 1024 * (scp + 1)],
                            ps[:], bqk[:, bcol : bcol + 1],
                        )

        # ---------------- phase 3: attention per head ----------------
        with ExitStack() as ao:
            att_pool = ao.enter_context(tc.tile_pool(name="att", bufs=1))
            attT = [att_pool.tile([128, S], BF, name=f"attT{h}", tag=f"attT{h}")
                    for h in range(H_LOC)]
            wo_pool = ao.enter_context(tc.tile_pool(name="wo", bufs=1))
            wo_sb = [wo_pool.tile([128, E], BF, name=f"wo{h}", tag=f"wo{h}")
                     for h in range(H_LOC)]
            for h in range(H_LOC):
                nc.sync.dma_start(
                    wo_sb[h][:], wo_d[128 * h : 128 * (h + 1), :]
                )

            with ExitStack() as ph:
                p_pool = ph.enter_context(tc.tile_pool(name="p", bufs=6))
                pt_pool = ph.enter_context(tc.tile_pool(name="pt", bufs=1))
                rs_pool = ph.enter_context(tc.tile_pool(name="rs", bufs=8))
                PT = pt_pool.tile([128, 4 * S], BF, name="PT", tag="PT")

                for h in range(H_LOC):
                    with nc.named_scope(f"attn{h}"):
                        for g in range(4):  # q-groups of 512
                            ptiles = []
                            for qs in range(4):
                                i = 4 * g + qs
                                L = 128 * (i + 1)
                                nch = (L + 1023) // 1024
                                p_i = p_pool.tile([128, S], BF, name="p", tag="p")
                                rs = rs_pool.tile([128, 4], FP, name="rs", tag="rs")
                                ptiles.append(p_i)
                                for c in range(nch):
                                    w = min(1024, L - 1024 * c)
                                    ps = psF.tile([128, 1024], FP,
                                                  name="psf", tag="psf")
                                    for hf in range(0, w, 512):
                                        hw = min(512, w - hf)
                                        nc.tensor.matmul(
                                            ps[:, hf : hf + hw],
                                            qT[h][:, 128 * i : 128 * (i + 1)],
                                            kT[h][:, 1024 * c + hf :
                                                   1024 * c + hf + hw],
                                            start=True, stop=True,
                                        )
                                    if c == nch - 1:
                                        nc.vector.tensor_add(
                                            ps[:, w - 128 : w],
                                            ps[:, w - 128 : w], cmask[:],
                                        )
                                    nc.scalar.activation(
                                        p_i[:, 1024 * c : 1024 * c + w],
                                        ps[:, :w], AF.Exp,
                                        accum_out=rs[:, c : c + 1],
                                    )
                                if nch > 1:
                                    nc.vector.tensor_add(
                                        rs[:, 0:1], rs[:, 0:1], rs[:, 1:2]
                                    )
                                nc.vector.reciprocal(rs[:, 2:3], rs[:, 0:1])
                                nc.vector.tensor_scalar_mul(
                                    p_i[:, :L], p_i[:, :L], rs[:, 2:3]
                                )
                            # P^T for the group, j-pairs
                            for jp in range(2 * g + 2):
                                pt = psT.tile([128, 1024], BF,
                                              name="pst", tag="pst")
                                for jj, base in ((2 * jp, 0), (2 * jp + 1, 512)):
                                    for qs in range(max(0, jj - 4 * g), 4):
                                        nc.tensor.transpose(
                                            pt[:, base + 128 * qs :
                                               base + 128 * (qs + 1)],
                                            ptiles[qs][:, 128 * jj :
                                                       128 * (jj + 1)],
                                            ident_bf[:],
                                        )
                                if jp < 2 * g:
                                    nc.vector.tensor_copy(
                                        PT[:, 1024 * jp : 1024 * (jp + 1)],
                                        pt[:],
                                    )
                                else:
                                    m0 = 2 * jp - 4 * g
                                    m1 = m0 + 1
                                    nc.vector.tensor_copy(
                                        PT[:, 1024 * jp + 128 * m0 :
                                           1024 * jp + 512],
                                        pt[:, 128 * m0 : 512],
                                    )
                                    nc.vector.tensor_copy(
                                        PT[:, 1024 * jp + 512 + 128 * m1 :
                                           1024 * (jp + 1)],
                                        pt[:, 512 + 128 * m1 : 1024],
                                    )
                            # PV for the group
                            po = psF.tile([128, 1024], FP, name="psf", tag="psf")
                            nkc = 4 * g + 4
                            for j in range(nkc):
                                qlo = max(0, 128 * (j - 4 * g))
                                nc.tensor.matmul(
                                    po[:, qlo:512],
                                    vsb[:, 512 * j + 128 * h :
                                        512 * j + 128 * (h + 1)],
                                    PT[:, 512 * j + qlo : 512 * (j + 1)],
                                    start=(j == 0), stop=(j == nkc - 1),
                                )
                            nc.vector.tensor_copy(
                                attT[h][:, 512 * g : 512 * (g + 1)],
                                po[:, :512],
                            )

            # ---------------- phase 4: output projection ----------------
            with ExitStack() as ph, nc.named_scope("outproj"):
                ostg = ph.enter_context(tc.tile_pool(name="ostg", bufs=4))
                for i in range(NB):
                    psums = [psF.tile([128, 1024], FP, name="psf", tag="psf")
                             for _ in range(2)]
                    for h in range(H_LOC):
                        ah = attT[h][:, 128 * i : 128 * (i + 1)]
                        first = h == 0
                        last = h == H_LOC - 1
                        for c in range(4):
                            nc.tensor.matmul(
                                psums[c // 2][:, 512 * (c % 2) :
                                              512 * (c % 2 + 1)],
                                ah, wo_sb[h][:, 512 * c : 512 * (c + 1)],
                                start=first, stop=last,
                            )
                    for half in range(2):
                        ot = ostg.tile([128, 1024], FP, name="ostg", tag="ostg")
                        if half == 0:
                            nc.vector.tensor_copy(ot[:], psums[half][:])
                        else:
                            nc.scalar.copy(ot[:], psums[half][:])
                        nc.sync.dma_start(
                            out[128 * i : 128 * (i + 1),
                                1024 * half : 1024 * (half + 1)],
                            ot[:],
                        )


_NC_CACHE = None


def _get_nc():
    global _NC_CACHE
    if _NC_CACHE is None:
        nc = bacc.Bacc(
            "TRN2",
            target_bir_lowering=False,
            debug=False,
            num_devices=1,
            enable_asserts=False,
        )
        _emit(nc)
        nc.compile()
        _NC_CACHE = nc
    return _NC_CACHE


def _bf(a):
    return np.ascontiguousarray(a.astype(ml_dtypes.bfloat16))


def make_in_maps(inX, W_qkv, b_qkv, W_out):
    xh = [_bf(inX[b]) for b in range(B)]
    per_hg = []
    for hg in range(HG):
        sl = slice(FLOC * hg, FLOC * (hg + 1))
        bqk = np.empty((128, 2 * H_LOC), dtype=np.float32)
        for h in range(H_LOC):
            f0 = FLOC * hg + 128 * h
            bqk[:, h] = b_qkv[f0 : f0 + 128] * SCALE
            bqk[:, H_LOC + h] = b_qkv[E + f0 : E + f0 + 128]
        per_hg.append(
            {
                "wq": _bf(W_qkv[:, 0:E][:, sl] * SCALE),
                "wk": _bf(W_qkv[:, E : 2 * E][:, sl]),
                "wv": _bf(W_qkv[:, 2 * E : 3 * E][:, sl]),
                "bqk": bqk,
                "wo": _bf(W_out[sl, :]),
            }
        )
    return [
        {"xh": xh[c // HG], **per_hg[c % HG]} for c in range(NCORES)
    ]


def kernel(inX, W_qkv, b_qkv, W_out, b_out):
    global LAST_EXEC_NS, LAST_RESULTS
    inX = np.asarray(inX, dtype=np.float32)
    W_qkv = np.asarray(W_qkv, dtype=np.float32)
    b_qkv = np.asarray(b_qkv, dtype=np.float32)
    W_out = np.asarray(W_out, dtype=np.float32)
    b_out = np.asarray(b_out, dtype=np.float32)

    nc = _get_nc()
    in_maps = make_in_maps(inX, W_qkv, b_qkv, W_out)

    kwargs = {}
    if PROFILE:
        kwargs = {"trace": True, "trace_cores": [0]}
    res = bass_utils.run_bass_kernel_spmd(
        nc, in_maps, core_ids=list(range(NCORES)), **kwargs
    )
    LAST_EXEC_NS = res.exec_time_ns
    LAST_RESULTS = res

    bias_full = (b_out + b_qkv[2 * E : 3 * E] @ W_out).astype(np.float32)
    out = np.empty((B, S, E), dtype=np.float32)
    for b in range(B):
        acc = res.results[HG * b + 0]["out"].astype(np.float64)
        for hg in range(1, HG):
            acc += res.results[HG * b + hg]["out"]
        out[b] = (acc + bias_full).astype(np.float32)
    return out
